# revision 52
# baseline (speedup 1.0000x reference)
"""Trainium2 Bass kernel for a transformer decoder layer (self-attn + cross-attn + FFN).

Contract: kernel(**inputs) takes the FULL unsharded inputs (as produced by
setup_inputs()) and returns the FULL outputs (out3, aw1, aw2), matching the
reference. Internally the work is sharded across 8 NeuronCores:
  core c -> batch b = c//4, row-group qi = c%4 owning the four strided
  128-row q-tiles {qi, qi+4, qi+8, qi+12} of the sequence.
Every core runs the SAME program; all per-core differences live in the data
slices passed via in_maps. No collectives are needed: K/V projections are
computed per-batch on each core (cheap), while attention rows, layernorms,
FFN rows and all outputs are disjoint per core.

Precision: matmul operands in bf16 (f32 PSUM accumulate), softmax in f32
without max subtraction (valid logits are bounded ~|8|), attention-weight
outputs written as f32 = bf16(exp) * f32(1/sum).

Scheduling: one long-lived PSUM pool with per-tag buffer counts shared by
every phase (no PSUM phase drains); SBUF pools are entered/exited manually
in dataflow order so that enc staging and MHA2 weight loads overlap MHA1.
"""

import os
import numpy as np

import concourse.bass as bass
import concourse.tile as tile
import concourse.mybir as mybir
from concourse import bacc
from concourse.bass_utils import run_bass_kernel_spmd
from concourse.masks import make_identity

F32 = mybir.dt.float32
BF16 = mybir.dt.bfloat16
AF = mybir.ActivationFunctionType
OP = mybir.AluOpType

B, S, DM, H, HID = 2, 2048, 512, 8, 2048
D = DM // H          # 64
NEG = -1e9
EPS = 1e-6
NC_COUNT = 8
NT = 4               # q-tiles per core (128 rows each)
P = 128
KC = DM // P         # 4 contraction chunks of 128 over DM
HC = HID // P        # 16 chunks over HID
ST = S // P          # 16 seq tiles


def _build():
    nc = bacc.Bacc("TRN2", target_bir_lowering=False, debug=False,
                   num_devices=NC_COUNT)

    dram = {}

    def din(name, shape):
        dram[name] = nc.dram_tensor(name, shape, F32, kind="ExternalInput").ap()

    def dout(name, shape):
        dram[name] = nc.dram_tensor(name, shape, F32, kind="ExternalOutput").ap()

    din("xb", [S, DM])          # x[b] (for K1/V1 projections)
    din("xq", [NT * P, DM])     # x[b, own rows] (Q1 + residual 1)
    din("encb", [S, DM])        # enc_output[b]
    din("bmask", [NT, P, 512])  # look-ahead mask boundary chunk per tile (raw 0/1)
    din("pm", [1, S])           # padding_mask row
    for l in ("1", "2"):
        for w in ("wq", "wk", "wv", "wo"):
            din(f"mha{l}_{w}", [DM, DM])
        for b_ in ("bq", "bk", "bv", "bo"):
            din(f"mha{l}_{b_}", [DM])
    din("ffn_w1", [DM, HID])
    din("ffn_b1", [HID])
    din("ffn_w2", [HID, DM])
    din("ffn_b2", [DM])
    for j in ("1", "2", "3"):
        din(f"ln{j}_g", [DM])
        din(f"ln{j}_b", [DM])

    dout("aw1_s", [H, NT, P, S])
    dout("aw2_s", [H, NT, P, S])
    dout("out3_s", [NT, P, DM])

    rep = int(os.environ.get("KERNEL_REP", "1"))
    with tile.TileContext(nc) as tc:
        if rep > 1:
            with tc.For_i(0, rep):
                _emit(tc, dram)
        else:
            _emit(tc, dram)
    nc.compile()
    return nc


def _emit(tc, dram):
    nc = tc.nc
    vec = nc.vector
    act = nc.scalar
    gp = nc.gpsimd

    import contextlib
    stack = contextlib.ExitStack()

    def pool(name, bufs=1):
        return stack.enter_context(tc.tile_pool(name=name, bufs=bufs))

    psA = stack.enter_context(tc.tile_pool(name="psA", bufs=1, space="PSUM"))
    cpool = pool("consts")
    opool = pool("carry")
    mpool = pool("misc", bufs=2)

    def t512():
        return psA.tile([P, 512], BF16, tag="t512", name="t512", bufs=2)

    def Lbuf():
        return psA.tile([P, 1024], F32, tag="L", name="L", bufs=2)

    def b512():
        return psA.tile([P, 512], F32, tag="b512", name="b512", bufs=2)

    def load_w_bf16(pl, name, kchunks, ncols):
        # DRAM [kchunks*128, ncols] f32 -> SBUF [128, kchunks, ncols] bf16
        t = pl.tile([P, kchunks, ncols], BF16, tag=f"w_{name}", name=name)
        gp.dma_start(t[:], dram[name].rearrange("(c p) n -> p c n", p=P))
        return t

    def load_bias_cols(pl, name, nch, scale=None):
        t = pl.tile([P, nch], F32, tag=f"b_{name}", name=name)
        nc.sync.dma_start(t[:], dram[name].rearrange("(c p) -> p c", p=P))
        if scale is not None:
            act.mul(t[:], t[:], scale)
        return t

    def bcast_row(pl, name, width=DM):
        row = pl.tile([1, width], F32, tag=f"r_{name}", name=f"r_{name}")
        nc.sync.dma_start(row[:], dram[name][:])
        t = pl.tile([P, width], F32, tag=f"bc_{name}", name=f"bc_{name}")
        gp.partition_broadcast(t[:], row[:])
        return t

    # ---------------- constants ----------------
    ident_bf = cpool.tile([P, P], BF16, tag="ident_bf")
    make_identity(nc, ident_bf[:])
    ones_bf = cpool.tile([1, P], BF16, tag="ones_bf")
    vec.memset(ones_bf[:], 1.0)

    def bias_row_bf(pl, name):
        t = pl.tile([1, DM], BF16, tag=f"br_{name}", name=f"br_{name}")
        gp.dma_start(t[:], dram[name][:])
        return t

    bv1_row = bias_row_bf(cpool, "mha1_bv")
    bo1_bc = bcast_row(cpool, "mha1_bo")
    bv2_row = bias_row_bf(cpool, "mha2_bv")
    bo2_bc = bcast_row(cpool, "mha2_bo")
    b2f_bc = bcast_row(cpool, "ffn_b2")
    g1_bc = bcast_row(cpool, "ln1_g")
    c1_bc = bcast_row(cpool, "ln1_b")
    g2_bc = bcast_row(cpool, "ln2_g")
    c2_bc = bcast_row(cpool, "ln2_b")
    g3_bc = bcast_row(cpool, "ln3_g")
    c3_bc = bcast_row(cpool, "ln3_b")

    out1 = opool.tile([P, NT, DM], F32, tag="out1")
    out1T = opool.tile([P, KC, NT * P], BF16, tag="out1T")
    out2 = opool.tile([P, NT, DM], F32, tag="out2")
    out2T = opool.tile([P, KC, NT * P], BF16, tag="out2T")

    def stage_transposed(pl, src_ap, nseq, tag):
        # [nseq, DM] f32 DRAM -> bf16 [128 dm-part, KC, nseq] (x^T layout)
        nst = nseq // P
        half = max(1, nst // 2)
        src3 = src_ap.rearrange("(t p) n -> p t n", p=P)
        dst = pl.tile([P, KC, nseq], BF16, tag=f"T_{tag}", name=f"T_{tag}")
        for hh in range(0, nst, half):
            hn = min(half, nst - hh)
            s_bf = pl.tile([P, half, DM], BF16, tag=f"stg_{tag}",
                           name=f"stg_{tag}")
            gp.dma_start(s_bf[:, :hn, :], src3[:, hh:hh + hn, :])
            for ti in range(hn):
                t = hh + ti
                ps = t512()
                for c in range(KC):
                    nc.tensor.transpose(ps[:, c * P:(c + 1) * P],
                                        s_bf[:, ti, c * P:(c + 1) * P],
                                        ident_bf[:])
                vec.tensor_copy(dst[:, :, t * P:(t + 1) * P], ps[:])
        return dst

    def transpose_512(src_f32_ap, dst, tcol):
        cb = mpool.tile([P, DM], BF16, tag="castT", name="castT")
        act.copy(cb[:], src_f32_ap)
        ps = t512()
        for c in range(KC):
            nc.tensor.transpose(ps[:, c * P:(c + 1) * P],
                                cb[:, c * P:(c + 1) * P], ident_bf[:])
        vec.tensor_copy(dst[:, :, tcol * P:(tcol + 1) * P], ps[:])

    def layernorm(lnp, z, g_bc, c_bc, out_ap):
        s = lnp.tile([P, 1], F32, tag="s", name="s")
        vec.reduce_sum(s[:], z[:], axis=mybir.AxisListType.X)
        mean = lnp.tile([P, 1], F32, tag="mean", name="mean")
        act.mul(mean[:], s[:], 1.0 / DM)
        sq = b512()
        ss = lnp.tile([P, 1], F32, tag="ss", name="ss")
        act.activation(sq[:], z[:], AF.Square, accum_out=ss[:])
        m2 = lnp.tile([P, 1], F32, tag="m2", name="m2")
        vec.tensor_mul(m2[:], mean[:], mean[:])
        var = lnp.tile([P, 1], F32, tag="var", name="var")
        act.mul(var[:], ss[:], 1.0 / DM)
        vec.tensor_sub(var[:], var[:], m2[:])
        vec.tensor_scalar_add(var[:], var[:], EPS)
        sd = lnp.tile([P, 1], F32, tag="sd", name="sd")
        act.sqrt(sd[:], var[:])
        rstd = lnp.tile([P, 1], F32, tag="rstd", name="rstd")
        vec.reciprocal(rstd[:], sd[:])
        vec.tensor_scalar(z[:], z[:], mean[:], rstd[:],
                          op0=OP.subtract, op1=OP.mult)
        vec.tensor_mul(z[:], z[:], g_bc[:])
        vec.tensor_add(out_ap, z[:], c_bc[:])

    # ---------------- projections for one attention layer ----------------
    def wtile(pl, name, kchunks, ncols, tag):
        t = pl.tile([P, kchunks, ncols], BF16, tag=tag, name=name)
        gp.dma_start(t[:], dram[name].rearrange("(c p) n -> p c n", p=P))
        return t

    def btile(pl, name, tag, scale=None):
        t = pl.tile([P, KC], F32, tag=tag, name=name)
        nc.sync.dma_start(t[:], dram[name].rearrange("(c p) -> p c", p=P))
        if scale is not None:
            act.mul(t[:], t[:], scale)
        return t

    def project(lname, wpool, kvT, qT_src, causal, pm_bf):
        wq = wtile(wpool, f"mha{lname}_wq", KC, DM, "w_q")
        wk = wtile(wpool, f"mha{lname}_wk", KC, DM, "w_k")
        wv = wtile(wpool, f"mha{lname}_wv", KC, DM, "w_v")
        wo = wtile(wpool, f"mha{lname}_wo", KC, DM, "w_o")
        bq_cols = btile(wpool, f"mha{lname}_bq", "b_q", 0.125)
        bk_cols = btile(wpool, f"mha{lname}_bk", "b_k")
        bv_row = wpool.tile([1, DM], BF16, tag="b_v", name=f"bv{lname}")
        gp.dma_start(bv_row[:], dram[f"mha{lname}_bv"][:])

        kt_tiles, v_tiles, qt_tiles = [], [], []
        for pr in range(4):   # head pairs
            # ---- K^T ----
            if causal:
                kt = wpool.tile([P, S], BF16, tag=f"kt_{pr}",
                                name=f"kt_{pr}")
                kt_tiles.append(kt)
            else:
                kth = [wpool.tile([65, S], BF16, tag=f"kt_{pr*2+hf}",
                                  name=f"kt_{pr*2+hf}") for hf in range(2)]
                kt_tiles.extend(kth)
            for c2 in range(2):
                kps = Lbuf()
                for c in range(KC):
                    for j in range(2):
                        nc.tensor.matmul(
                            kps[:, j * 512:(j + 1) * 512],
                            wk[:, c, pr * P:(pr + 1) * P],
                            kvT[:, c, c2 * 1024 + j * 512:
                                c2 * 1024 + (j + 1) * 512],
                            start=(c == 0), stop=(c == KC - 1))
                cs = slice(c2 * 1024, (c2 + 1) * 1024)
                if causal:
                    vec.tensor_scalar_add(kt[:, cs], kps[:],
                                          bk_cols[:, pr:pr + 1])
                else:
                    kpair = mpool.tile([P, 1024], BF16, tag="kpair",
                                       name="kpair")
                    vec.tensor_scalar_add(kpair[:], kps[:],
                                          bk_cols[:, pr:pr + 1])
                    for hf in range(2):
                        nc.sync.dma_start(kth[hf][0:64, cs],
                                          kpair[hf * 64:(hf + 1) * 64, :])
            if not causal:
                for hf in range(2):
                    nc.sync.dma_start(kth[hf][64:65, :], pm_bf[:])
            # ---- V ----
            vt = wpool.tile([P, ST, P], BF16, tag=f"v_{pr}", name=f"v_{pr}")
            v_tiles.append(vt)
            for st4 in range(ST // 4):
                vps = b512()
                for q4 in range(4):
                    st = st4 * 4 + q4
                    for c in range(KC):
                        nc.tensor.matmul(vps[:, q4 * P:(q4 + 1) * P],
                                         kvT[:, c, st * P:(st + 1) * P],
                                         wv[:, c, pr * P:(pr + 1) * P],
                                         start=(c == 0), stop=False)
                    nc.tensor.matmul(vps[:, q4 * P:(q4 + 1) * P],
                                     ones_bf[:],
                                     bv_row[:, pr * P:(pr + 1) * P],
                                     start=False, stop=True)
                act.copy(vt[:, st4 * 4:(st4 + 1) * 4, :], vps[:])
            # ---- Q^T (own rows), pre-scaled by 1/8 ----
            qps = b512()
            for c in range(KC):
                nc.tensor.matmul(qps[:], wq[:, c, pr * P:(pr + 1) * P],
                                 qT_src[:, c, :],
                                 start=(c == 0), stop=(c == KC - 1))
            if causal:
                qt = wpool.tile([P, NT * P], BF16, tag=f"qt_{pr}",
                                name=f"qt_{pr}")
                vec.tensor_scalar(qt[:], qps[:], 0.125, bq_cols[:, pr:pr + 1],
                                  op0=OP.mult, op1=OP.add)
                qt_tiles.append(qt)
            else:
                qpair = mpool.tile([P, NT * P], BF16, tag="qpair",
                                   name="qpair")
                vec.tensor_scalar(qpair[:], qps[:], 0.125,
                                  bq_cols[:, pr:pr + 1],
                                  op0=OP.mult, op1=OP.add)
                for hf in range(2):
                    qt = wpool.tile([65, NT * P], BF16, tag=f"qt_{pr*2+hf}",
                                    name=f"qt_{pr*2+hf}")
                    nc.sync.dma_start(qt[0:64, :],
                                      qpair[hf * 64:(hf + 1) * 64, :])
                    vec.memset(qt[64:65, :], 1.0)
                    qt_tiles.append(qt)
        return wo, kt_tiles, v_tiles, qt_tiles

    # ---------------- attention loop for one layer ----------------
    def attend(lname, sm, lnp, wo, kt_tiles, v_tiles, qt_tiles, res_of_t,
               bo_bc, g_bc, cst_bc, aw_out, causal, out_sb, bm=None):
        for t in range(NT):
            nv = 512 * (t + 1) if causal else S
            nb = nv // P
            ctxT = sm.tile([P, KC, P], BF16, tag="ctxT", name="ctxT")
            for h in range(H):
                pr, hf = h // 2, h % 2
                if causal:
                    lhsq = qt_tiles[pr][hf * 64:(hf + 1) * 64,
                                        t * P:(t + 1) * P]
                else:
                    lhsq = qt_tiles[h][:, t * P:(t + 1) * P]
                E = sm.tile([P, S], BF16, tag="E", name="E", bufs=3)
                scs = []
                nch = (nv + 1023) // 1024
                for ch in range(nch):
                    clen = min(1024, nv - ch * 1024)
                    L = Lbuf()
                    for j in range(clen // 512):
                        off = ch * 1024 + j * 512
                        if causal:
                            rhsk = kt_tiles[pr][hf * 64:(hf + 1) * 64,
                                                off:off + 512]
                        else:
                            rhsk = kt_tiles[h][:, off:off + 512]
                        nc.tensor.matmul(L[:, j * 512:(j + 1) * 512],
                                         lhsq, rhsk, start=True, stop=True)
                    if causal and t * 512 // 1024 == ch:
                        boff = t * 512 % 1024
                        vec.tensor_add(L[:, boff:boff + 512],
                                       L[:, boff:boff + 512], bm[:, t, :])
                    sc = sm.tile([P, 1], F32, tag="sc", name="sc")
                    act.activation(E[:, ch * 1024:ch * 1024 + clen],
                                   L[:, :clen], AF.Exp, accum_out=sc[:])
                    scs.append(sc)
                if nch == 1:
                    ssum = scs[0]
                else:
                    ssum = sm.tile([P, 1], F32, tag="ssum", name="ssum")
                    vec.tensor_add(ssum[:], scs[0][:], scs[1][:])
                rs = sm.tile([P, 1], F32, tag="rs", name="rs")
                vec.reciprocal(rs[:], ssum[:])
                if not causal:
                    # f32 aw from UNnormalized E (avoids extra bf16 rounding)
                    for ch in range(nch):
                        clen = min(1024, nv - ch * 1024)
                        awf = sm.tile([P, 1024], F32, tag="awf", name="awf")
                        eng = (vec, vec, vec, act)[h % 4]
                        if eng is act:
                            act.activation(
                                awf[:, :clen],
                                E[:, ch * 1024:ch * 1024 + clen],
                                AF.Identity, scale=rs[:])
                        else:
                            eng.tensor_scalar_mul(
                                awf[:, :clen],
                                E[:, ch * 1024:ch * 1024 + clen], rs[:])
                        nc.sync.dma_start(
                            aw_out[h, t, :, ch * 1024:ch * 1024 + clen],
                            awf[:, :clen])
                vec.tensor_scalar_mul(E[:, :nv], E[:, :nv], rs[:])
                if causal:
                    # f32 aw output via SWDGE casting DMA from normalized E
                    gp.dma_start(aw_out[h, t, :, :nv], E[:, :nv])
                # ---- P^T via PE, then P @ V ----
                if hf == 0:
                    cpair = psA.tile([P, P], F32, tag="b512", name="cps",
                                     bufs=2)
                cps = cpair[hf * 64:(hf + 1) * 64, :]
                for k4 in range((nb + 3) // 4):
                    tp = t512()
                    kn = min(4, nb - k4 * 4)
                    for q4 in range(kn):
                        kc = k4 * 4 + q4
                        nc.tensor.transpose(tp[:, q4 * P:(q4 + 1) * P],
                                            E[:, kc * P:(kc + 1) * P],
                                            ident_bf[:])
                    pts = sm.tile([P, 512], BF16, tag="pts", name="pts", bufs=4)
                    vec.tensor_copy(pts[:, :kn * P], tp[:, :kn * P])
                    for q4 in range(kn):
                        kc = k4 * 4 + q4
                        nc.tensor.matmul(
                            cps,
                            v_tiles[pr][:, kc, hf * 64:(hf + 1) * 64],
                            pts[:, q4 * P:(q4 + 1) * P],
                            start=(kc == 0), stop=(kc == nb - 1))
                if hf == 1:
                    act.copy(ctxT[:, pr, :], cpair[:])
            # ---- Wo + residual + LN ----
            ops = b512()
            for c in range(KC):
                nc.tensor.matmul(ops[:], ctxT[:, c, :], wo[:, c, :],
                                 start=(c == 0), stop=(c == KC - 1))
            z = lnp.tile([P, DM], F32, tag="z", name="z")
            vec.tensor_add(z[:], ops[:], res_of_t(t))
            vec.tensor_add(z[:], z[:], bo_bc[:])
            layernorm(lnp, z, g_bc, cst_bc, out_sb[:, t, :])

    # ================= emission in dataflow order =================
    pl1 = tc.alloc_tile_pool(name="l1misc", bufs=1)
    xq_res = pl1.tile([P, NT, DM], F32, tag="xq_res")
    nc.sync.dma_start(xq_res[:],
                      dram["xq"].rearrange("(t p) n -> p t n", p=P))
    bm = pl1.tile([P, NT, 512], BF16, tag="bmask")
    gp.dma_start(bm[:], dram["bmask"].rearrange("t p n -> p t n"))
    vec.tensor_scalar_mul(bm[:], bm[:], NEG)

    pw = tc.alloc_tile_pool(name="wkv", bufs=1)   # shared by both layers
    px = tc.alloc_tile_pool(name="xstage", bufs=1)
    xT = stage_transposed(px, dram["xb"], S, "x")
    xqT = stage_transposed(px, dram["xq"], NT * P, "xq")
    wo1, kt1, v1, qt1 = project("1", pw, xT, xqT, True, None)
    px.release()          # free xT/xqT space

    # enc staging: emitted now so it overlaps MHA1 attention
    penc = tc.alloc_tile_pool(name="encstage", bufs=1)
    pm_bf = penc.tile([1, S], BF16, tag="pm_bf")
    gp.dma_start(pm_bf[:], dram["pm"][:])
    vec.tensor_scalar_mul(pm_bf[:], pm_bf[:], NEG)
    encT = stage_transposed(penc, dram["encb"], S, "enc")

    sm1 = tc.alloc_tile_pool(name="sm_1", bufs=2)
    ln1 = tc.alloc_tile_pool(name="ln_1", bufs=2)
    attend("1", sm1, ln1, wo1, kt1, v1, qt1, lambda t: xq_res[:, t, :],
           bo1_bc, g1_bc, c1_bc, dram["aw1_s"], True, out1, bm=bm)
    ln1.release()
    sm1.release()

    for t in range(NT):
        transpose_512(out1[:, t, :], out1T, t)

    # MHA2 projections reuse the SAME pool tags (slot rotation = reuse)
    wo2, kt2, v2, qt2 = project("2", pw, encT, out1T, False, pm_bf)
    penc.release()        # free encT

    sm2 = tc.alloc_tile_pool(name="sm_2", bufs=2)
    ln2 = tc.alloc_tile_pool(name="ln_2", bufs=2)
    attend("2", sm2, ln2, wo2, kt2, v2, qt2, lambda t: out1[:, t, :],
           bo2_bc, g2_bc, c2_bc, dram["aw2_s"], False, out2)
    ln2.release()
    sm2.release()
    pw.release()

    for t in range(NT):
        transpose_512(out2[:, t, :], out2T, t)

    # ================= FFN + LN3 =================
    fp = tc.alloc_tile_pool(name="ffn", bufs=1)
    w1_sb = load_w_bf16(fp, "ffn_w1", KC, HID)
    w2_sb = load_w_bf16(fp, "ffn_w2", HC, DM)
    b1f = load_bias_cols(fp, "ffn_b1", HC)
    hT = fp.tile([P, HC, NT * P], BF16, tag="hT")
    for hc in range(HC):
        hp = b512()
        for c in range(KC):
            nc.tensor.matmul(hp[:], w1_sb[:, c, hc * P:(hc + 1) * P],
                             out2T[:, c, :],
                             start=(c == 0), stop=(c == KC - 1))
        act.activation(hT[:, hc, :], hp[:], AF.Relu, bias=b1f[:, hc:hc + 1])
    lnp3 = tc.alloc_tile_pool(name="ln3", bufs=2)
    for t in range(NT):
        op3 = b512()
        for hc in range(HC):
            nc.tensor.matmul(op3[:], hT[:, hc, t * P:(t + 1) * P],
                             w2_sb[:, hc, :],
                             start=(hc == 0), stop=(hc == HC - 1))
        z = lnp3.tile([P, DM], F32, tag="z3", name="z3")
        vec.tensor_add(z[:], op3[:], out2[:, t, :])
        vec.tensor_add(z[:], z[:], b2f_bc[:])
        o3 = lnp3.tile([P, DM], F32, tag="o3", name="o3")
        layernorm(lnp3, z, g3_bc, c3_bc, o3[:])
        nc.sync.dma_start(dram["out3_s"][t], o3[:])
    lnp3.release()
    fp.release()
    pl1.release()

    stack.close()


_NC_CACHE = None


def _get_nc():
    global _NC_CACHE
    if _NC_CACHE is None:
        _NC_CACHE = _build()
    return _NC_CACHE


def _make_in_maps(inputs):
    f = np.float32
    x = np.asarray(inputs["x"], f)
    enc = np.asarray(inputs["enc_output"], f)
    lam = np.asarray(inputs["look_ahead_mask"], f)[0, 0]   # [S, S]
    pm = np.ascontiguousarray(np.asarray(inputs["padding_mask"],
                                         f).reshape(1, S))

    shared = {}
    for l in ("1", "2"):
        for w in ("wq", "wk", "wv", "wo", "bq", "bk", "bv", "bo"):
            shared[f"mha{l}_{w}"] = np.ascontiguousarray(
                np.asarray(inputs[f"mha{l}_{w}"], f))
    for k in ("ffn_w1", "ffn_b1", "ffn_w2", "ffn_b2",
              "ln1_g", "ln1_b", "ln2_g", "ln2_b", "ln3_g", "ln3_b"):
        shared[k] = np.ascontiguousarray(np.asarray(inputs[k], f))

    in_maps = []
    for c in range(NC_COUNT):
        b, qi = c // 4, c % 4
        rows = np.concatenate([np.arange(128 * (qi + 4 * t),
                                         128 * (qi + 4 * t + 1))
                               for t in range(NT)])
        bmask = np.stack([lam[128 * (qi + 4 * t):128 * (qi + 4 * t + 1),
                              512 * t:512 * (t + 1)] for t in range(NT)])
        m = dict(shared)
        m["xb"] = np.ascontiguousarray(x[b])
        m["xq"] = np.ascontiguousarray(x[b][rows])
        m["encb"] = np.ascontiguousarray(enc[b])
        m["bmask"] = np.ascontiguousarray(bmask)
        m["pm"] = pm
        in_maps.append(m)
    return in_maps


def kernel(**inputs):
    nc = _get_nc()
    in_maps = _make_in_maps(inputs)
    res = run_bass_kernel_spmd(nc, in_maps, core_ids=list(range(NC_COUNT)))
    f = np.float32
    out3 = np.empty((B, S, DM), f)
    aw1 = np.empty((B, H, S, S), f)
    aw2 = np.empty((B, H, S, S), f)
    for c in range(NC_COUNT):
        b, qi = c // 4, c % 4
        r = res.results[c]
        for t in range(NT):
            g = qi + 4 * t
            sl = slice(128 * g, 128 * (g + 1))
            aw1[b, :, sl, :] = r["aw1_s"][:, t]
            aw2[b, :, sl, :] = r["aw2_s"][:, t]
            out3[b, sl, :] = r["out3_s"][t]
    return out3, aw1, aw2
